# revision 71
# baseline (speedup 1.0000x reference)
"""BiModal attention kernel for Trainium2 (8 NeuronCores, data-parallel over batch).

Per core (one batch b): x, y: [2048, 128] fp32.
  S = x @ y.T                    (float32r matmuls, [2048, 2048])
  E = exp(S)                     (unshifted; softmax is shift-invariant and
                                  |S| <~ 67 so exp stays in fp32/bf16 range)
  a1 = (E @ y) / rowsum(E) * x
  a2 = (E.T @ x) / colsum(E) * y
  out = concat([a1, a2], -1)     ([2048, 256])

Layout: rows are relabeled r = 16*p + b (p = SBUF partition, b = block index)
so every DRAM transfer is contiguous per partition; the relabeling is applied
consistently to s and t everywhere, so the math is unchanged.

v8: uniform software pipeline. One slot per (panel ct, row block i), 32 slots.
Per slot: PE runs S (2x512 f32r) + o2 (2x512 bf16, one slot behind exp) + a
2-tb o1 chunk (512 s-cols); ACT runs exp (1 slot behind S) with l1 accum;
sync issues the E->ET xbar right after exp; DVE reduces that xbar's 8x128 ET
slice into l2 partials one slot later and handles PSUM drains; GpSimd does
bf16 operand copies, all output gating, and the output stores. o1 quarter
(p,j) runs as 2-tb chunks in slots 16p+4j+5..+8 (ET available 2 slots after
the quarter's last xbar), drain at +9, except (1,3) which runs in the tail
as two 8-tb x 256-col slices chasing the last xbars. Epilogue transposes are
quarter-granular ([128,512] via DMA xbar) and split across sync (o1 side)
and ACT (o2 side). Stores are combined [128, 4, 256] (o1|o2 halves gated
into one staging tile) so DRAM lines are 1KB contiguous.
PSUM: sA 2x[128,1024] (S rotation), o2_ps [128,1024] (panel accum), c_ps
[128,1024] (o1 quarter halves, alternating); prologue transposes stage
through all four before their owners need them.
"""
import sys

sys.path.insert(0, "/opt/trn_rl_repo")

import os
import numpy as np

import concourse.bass as bass
import concourse.mybir as mybir
import concourse.tile as tile
from concourse.tile_rust import add_dep_helper
from concourse import bacc
from concourse.bass_utils import run_bass_kernel_spmd
from concourse.masks import make_identity

f32 = mybir.dt.float32
f32r = mybir.dt.float32r
bf16 = mybir.dt.bfloat16

B = 8
S = 2048
D = 128
P = 128
NB = S // P          # 16 row/col blocks
NP = 2               # column panels
PW = S // NP         # panel width (1024)
PB = PW // P         # blocks per panel (8)

_NC_CACHE = None
LAST_EXEC_NS = None


def _build_program(nc):
    x_d = nc.dram_tensor("x", [S, D], f32, kind="ExternalInput").ap()
    y_d = nc.dram_tensor("y", [S, D], f32, kind="ExternalInput").ap()
    out_d = nc.dram_tensor("out", [S, 2 * D], f32, kind="ExternalOutput").ap()

    x_dv = x_d.rearrange("(p b) d -> p b d", p=P)      # [128, 16, 128]
    y_dv = y_d.rearrange("(p b) d -> p b d", p=P)
    out_dv = out_d.rearrange("(p b) c -> p b c", p=P)  # [128, 16, 256]

    Exp = mybir.ActivationFunctionType.Exp
    MUL = mybir.AluOpType.mult
    ADD = mybir.AluOpType.add
    AX = mybir.AxisListType.X

    last = {}

    def seq(key, inst):
        prev = last.get(key)
        if prev is not None:
            add_dep_helper(inst.ins, prev.ins, sync=False, reason="order")
        last[key] = inst
        return inst

    def xdep(inst, *writers):
        # Explicit RAW dep on DMA-transpose writes: the tile framework does
        # not track dma_start_transpose OUTPUT regions, so every consumer of
        # a transposed tile must be ordered manually.
        for w in writers:
            if w is None:
                continue
            for wi in (w if isinstance(w, list) else [w]):
                add_dep_helper(inst.ins, wi.ins, sync=True, reason="xbar-out")
        return inst

    with tile.TileContext(nc) as tc:
        with (
            tc.tile_pool(name="sb", bufs=1) as sb,
            tc.tile_pool(name="stg", bufs=4) as stg,
            tc.tile_pool(name="ps", bufs=1, space="PSUM") as ps,
        ):
            # ---- persistent SBUF tensors ----
            x_sb = sb.tile([P, NB, D], f32, tag="x_sb")
            y_sb = sb.tile([P, NB, D], f32, tag="y_sb")
            xT = sb.tile([P, NB, P], f32r, tag="xT")       # [d, sb, sp]
            yT = sb.tile([P, NB, P], f32r, tag="yT")       # [d, tb, tp]
            x_hi = sb.tile([P, NB, D], bf16, tag="x_hi")
            y_hi = sb.tile([P, NB, D], bf16, tag="y_hi")
            E = sb.tile([P, NB, S], bf16, tag="E")         # [sp, sb, t-pos]
            ET = sb.tile([P, NB, S], bf16, tag="ET")       # [tp, tb, s-pos]
            o1T_sb = sb.tile([P, S], bf16, tag="o1T")      # [d, s-pos]
            o2T_sb = sb.tile([P, S], bf16, tag="o2T")      # [d, t-pos]
            o1N = sb.tile([P, NB, D], bf16, tag="o1N")     # [sp, sb, d]
            o2N = sb.tile([P, NB, D], bf16, tag="o2N")     # [tp, tb, d]
            ident = sb.tile([P, P], f32, tag="ident")
            l1p = sb.tile([P, 2 * NB], f32, tag="l1p")     # [sp, 2*i+ct]
            l1c = sb.tile([P, NB], f32, tag="l1c")
            r1 = sb.tile([P, NB], f32, tag="r1")
            l2p = sb.tile([P, NB, 4], f32, tag="l2p")      # [tp, tb, group]
            l2t1 = sb.tile([P, PB, P], bf16, tag="l2t1")   # TT-tree tmps
            l2t2 = sb.tile([P, PB, P], bf16, tag="l2t2")
            l2 = sb.tile([P, NB], f32, tag="l2")
            r2 = sb.tile([P, NB], f32, tag="r2")

            make_identity(nc, ident[:])

            # ---- PSUM tiles (8 banks) ----
            sA = [ps.tile([P, PW], f32, tag="A0", name="sA0"),
                  ps.tile([P, PW], f32, tag="A1", name="sA1")]
            o2_ps = ps.tile([P, PW], f32, tag="B", name="o2_ps")
            c_ps = ps.tile([P, PW], f32, tag="C", name="c_ps")

            yT_f = yT[:].rearrange("p b d -> p (b d)")

            # ---- loads on 3 queue rings (~100GB/s each). The sync ring must
            # stay short: its later transfers would delay the first E-xbars.
            # First-S deps (x0, y0-7) land first.
            # first-needed chunks via gpsimd SWDGE (its queue is live at t~0;
            # the sync/scalar sequencers take ~7us to boot); the sync ring
            # stays clear so the first E-xbar transfer isn't queued behind
            # load traffic
            seq("sp", nc.sync.dma_start(x_sb[:, 0:4], x_dv[:, 0:4]))
            seq("sp", nc.sync.dma_start(y_sb[:, 0:4], y_dv[:, 0:4]))
            seq("act", nc.scalar.dma_start(y_sb[:, 4:8], y_dv[:, 4:8]))
            seq("act", nc.scalar.dma_start(x_sb[:, 4:8], x_dv[:, 4:8]))
            seq("act", nc.scalar.dma_start(y_sb[:, 8:16], y_dv[:, 8:16]))
            seq("act", nc.scalar.dma_start(x_sb[:, 8:16], x_dv[:, 8:16]))

            # ---- emission helpers ----
            def ptr(v_sb, b, pslice):
                seq("pe", nc.tensor.transpose(pslice, v_sb[:, b, :], ident[:]))

            def drain4(q, dst, b0, src, n=4):
                # [128, n*128] PSUM -> SBUF f32r
                eng = {"act": nc.scalar, "dve": nc.vector}[q]
                if q == "act":
                    seq(q, eng.copy(dst[:, b0:b0 + n, :], src))
                else:
                    seq(q, eng.tensor_scalar_add(dst[:, b0:b0 + n, :], src, 0.0))

            def conv(q, dst, src, b0, n):
                # f32 -> bf16 operand copies; "gp" uses the Pool engine's
                # casting DMA (software DGE) to keep DVE/ACT free
                if q == "gp":
                    seq(q, nc.gpsimd.dma_start(
                        dst[:, b0:b0 + n, :], src[:, b0:b0 + n, :]))
                elif q == "act":
                    seq(q, nc.scalar.copy(dst[:, b0:b0 + n, :],
                                          src[:, b0:b0 + n, :]))
                else:
                    seq(q, nc.vector.tensor_scalar_add(
                        dst[:, b0:b0 + n, :], src[:, b0:b0 + n, :], 0.0))

            def s_block(ct, i):
                c0 = ct * PW
                for q in range(2):
                    seq("pe", nc.tensor.matmul(
                        sA[i % 2][:, q * 512:(q + 1) * 512], xT[:, i, :],
                        yT_f[:, c0 + q * 512:c0 + (q + 1) * 512],
                        start=True, stop=True))

            def exp_block(ct, i):
                c0 = ct * PW
                idx = 2 * i + ct
                seq("act", nc.scalar.activation(
                    E[:, i, c0:c0 + PW], sA[i % 2][:], Exp,
                    accum_out=l1p[:, idx:idx + 1]))

            xbar_inst = {}
            epi1_inst = {}
            epi2_inst = {}

            def xbar(ct, i):
                c0 = ct * PW
                xbar_inst[(ct, i)] = seq("sp", nc.sync.dma_start_transpose(
                    ET[:, ct * PB:(ct + 1) * PB, i * P:(i + 1) * P],
                    E[:, i, c0:c0 + PW]))

            def o2_block(ct, i):
                c0 = ct * PW
                for q in range(2):
                    seq("pe", nc.tensor.matmul(
                        o2_ps[:, q * 512:(q + 1) * 512], x_hi[:, i, :],
                        E[:, i, c0 + q * 512:c0 + (q + 1) * 512],
                        start=(i == 0), stop=(i == NB - 1)))

            def o1_chunk(p, j, tbs):
                # o1 quarter (panel p, s-cols 512j..512j+512): contract tbs.
                # NOTE: start=True zeroes the whole 2KB psum bank, so each
                # bank half carries exactly one open group at a time.
                # Only the quarter's first matmul needs the xbar deps: the
                # rest follow in PE queue order.
                half = j % 2
                cb = c_ps[:, half * 512:half * 512 + 512]
                for tb in tbs:
                    inst = seq("pe", nc.tensor.matmul(
                        cb, y_hi[:, p * PB + tb, :],
                        ET[:, p * PB + tb, j * 512:(j + 1) * 512],
                        start=(tb == 0), stop=(tb == PB - 1)))
                    if tb == 0:
                        xdep(inst, *(xbar_inst.get((p, 4 * j + q))
                                     for q in range(4)))

            def o1_drain(p, j, cols=None):
                # both panels drain on DVE (an ACT copy here would stall the
                # exp cadence, which has no slack). cols slices the drain
                # (safe: reads only; no psum zero-region hazard)
                half = j % 2
                lo, hi = (0, 512) if cols is None else cols
                cb = c_ps[:, half * 512 + lo:half * 512 + hi]
                dst = o1T_sb[:, j * 512 + lo:j * 512 + hi]
                with nc.allow_low_precision(reason="bf16 o1 staging"):
                    if p == 0:
                        seq("dve", nc.vector.tensor_scalar_add(dst, cb, 0.0))
                    else:
                        seq("dve", nc.vector.tensor_tensor(dst, cb, dst,
                                                           op=ADD))

            def l2_tt(ct, dst, ia, ib):
                # pairwise bf16 add of two landed ET slices (DVE 2x mode);
                # exact sum association, one bf16 rounding per level
                tbs = slice(ct * PB, (ct + 1) * PB)
                with nc.allow_low_precision(reason="bf16 l2 tree"):
                    if isinstance(ia, int):
                        inst = seq("dve", nc.vector.tensor_tensor(
                            dst[:], ET[:, tbs, ia * P:(ia + 1) * P],
                            ET[:, tbs, ib * P:(ib + 1) * P], op=ADD))
                        xdep(inst, xbar_inst.get((ct, ia)),
                             xbar_inst.get((ct, ib)))
                    else:
                        seq("dve", nc.vector.tensor_tensor(
                            dst[:], ia[:], ib[:], op=ADD))

            def l2_red(ct, k):
                # reduce the group's tree root -> l2p[:, tbs, k]
                seq("dve", nc.vector.tensor_reduce(
                    l2p[:, ct * PB:(ct + 1) * PB, k:k + 1],
                    l2t1[:], axis=AX, op=ADD))

            def l2_merge(p):
                seq("dve", nc.vector.tensor_reduce(
                    l2[:, p * PB:(p + 1) * PB],
                    l2p[:, p * PB:(p + 1) * PB, :], axis=AX, op=ADD))
                seq("dve", nc.vector.reciprocal(r2[:, p * PB:(p + 1) * PB],
                                                l2[:, p * PB:(p + 1) * PB]))

            def l1_chunk(k):
                src = l1p[:, 8 * k:8 * k + 8].rearrange("p (i c) -> p i c", c=2)
                seq("dve", nc.vector.tensor_reduce(l1c[:, 4 * k:4 * k + 4], src,
                                                   axis=AX, op=ADD))
                seq("dve", nc.vector.reciprocal(r1[:, 4 * k:4 * k + 4],
                                                l1c[:, 4 * k:4 * k + 4]))

            def epi_xbar1(j, h=None, q="sp"):
                # o1T quarter [128, 512] -> o1N blocks 4j..4j+3; h selects a
                # [128, 256] half (2 blocks) for the pipelined tail
                eng = {"act": nc.scalar, "sp": nc.sync}[q]
                if h is None:
                    inst = seq(q, eng.dma_start_transpose(
                        o1N[:, 4 * j:4 * j + 4, :],
                        o1T_sb[:, j * 512:(j + 1) * 512]))
                    epi1_inst[(j, 0)] = epi1_inst[(j, 1)] = inst
                else:
                    inst = seq(q, eng.dma_start_transpose(
                        o1N[:, 4 * j + 2 * h:4 * j + 2 * h + 2, :],
                        o1T_sb[:, j * 512 + 256 * h:j * 512 + 256 * h + 256]))
                    epi1_inst[(j, h)] = inst

            def epi_xbar2(half, q="act"):
                # o2T half [128, 1024] -> o2N blocks 8h..8h+7
                eng = {"act": nc.scalar, "sp": nc.sync}[q]
                epi2_inst[half] = seq(q, eng.dma_start_transpose(
                    o2N[:, half * PB:(half + 1) * PB, :],
                    o2T_sb[:, half * PW:(half + 1) * PW]))

            st_tiles = {}

            def gate(which, blks):
                # gate blocks of o1 (which=1) or o2 (which=2) into staging
                k = blks[0] // 4
                if k not in st_tiles:
                    st_tiles[k] = stg.tile([P, 4, 2 * D], f32, tag="st",
                                           name=f"st{k}")
                st = st_tiles[k]
                srcN = o1N if which == 1 else o2N
                rcp = r1 if which == 1 else r2
                gsrc = x_sb if which == 1 else y_sb
                col0 = 0 if which == 1 else D
                for blk in blks:
                    inst = seq("dve", nc.vector.scalar_tensor_tensor(
                        st[:, blk % 4, col0:col0 + D], srcN[:, blk, :],
                        rcp[:, blk:blk + 1], gsrc[:, blk, :],
                        op0=MUL, op1=MUL))
                    if which == 1:
                        xdep(inst, epi1_inst.get((blk // 4, (blk % 4) // 2)))
                    else:
                        xdep(inst, epi2_inst.get(blk // 8))

            def store(k):
                seq("gp", nc.gpsimd.dma_start(
                    out_dv[:, 4 * k:4 * k + 4, :], st_tiles[k][:]))

            # ---- prologue: minimal set for S(0,0) = xT[0] + yT[0:8];
            # everything else staged just-in-time during early slots ----
            ptr(x_sb, 0, sA[0][:, 0:P])          # x0 -> A0.lo[0:128]
            for b in range(1, 4):                # x1-3 -> A0.hi[512:896]
                ptr(x_sb, b, sA[0][:, (b + 3) * P:(b + 4) * P])
            for b in range(4):                   # y0-3 -> B[0:512]
                ptr(y_sb, b, o2_ps[:, b * P:(b + 1) * P])
            for b in range(4, 8):                # y4-7 -> A1[0:512]
                ptr(y_sb, b, sA[1][:, (b - 4) * P:(b - 3) * P])
            drain4("act", xT, 0, sA[0][:, 0:P], n=1)
            drain4("act", yT, 0, o2_ps[:, 0:512])
            drain4("act", yT, 4, sA[1][:, 0:512])
            drain4("dve", xT, 1, sA[0][:, 512:896], n=3)
            # bf16 copies (gpsimd casting DMAs), first-needed first
            conv("gp", x_hi, x_sb, 0, 4)
            conv("gp", y_hi, y_sb, 0, 4)
            conv("gp", x_hi, x_sb, 4, 4)
            conv("gp", y_hi, y_sb, 4, 4)
            conv("gp", x_hi, x_sb, 8, 8)
            conv("gp", y_hi, y_sb, 8, 8)

            # late transposes (blocks 8-15) through C halves, slots 4-7
            # (ordered to match when their load chunks land)
            def late_ptr(g):
                v_sb, b0, dst, half = (
                    (y_sb, 8, yT, 0), (y_sb, 12, yT, 0),
                    (x_sb, 8, xT, 1), (x_sb, 12, xT, 1))[g]
                for b in range(b0, b0 + 4):
                    ptr(v_sb, b, c_ps[:, half * 512 + (b - b0) * P:
                                      half * 512 + (b - b0 + 1) * P])
                drain4("dve", dst, b0, c_ps[:, half * 512:half * 512 + 512])

            # o1 chunk slot map: (p, j) occupies slots 16p+4j+6 .. +9,
            # 2 tbs per slot (1 slot of margin past the quarter's last xbar);
            # (1,2) spills 2 chunks to the tail, (1,3) runs fully in the tail.
            o1_sched = {}
            for p_ in range(2):
                for j_ in range(4):
                    if (p_, j_) == (1, 3):
                        continue
                    base = 16 * p_ + 4 * j_ + 6
                    for c_ in range(4):
                        g_ = base + c_
                        if g_ >= 32:
                            continue
                        o1_sched.setdefault(g_, []).append(
                            (p_, j_, (2 * c_, 2 * c_ + 1)))
            o1_drain_sched = {16 * p_ + 4 * j_ + 10: (p_, j_)
                              for p_ in range(2) for j_ in range(4)
                              if 16 * p_ + 4 * j_ + 10 < 32}

            # ---- main loop: 32 uniform slots ----
            for g in range(32):
                ct, i = divmod(g, 16)
                s_block(ct, i)
                if g == 16:
                    # close panel-0 o2 and drain it BEFORE o2_block(1, 0)
                    # opens the new accumulation group on the same banks;
                    # drain in bank halves so the WAR releases per-bank
                    o2_block(0, NB - 1)
                    with nc.allow_low_precision(reason="bf16 o2 staging"):
                        seq("dve", nc.vector.tensor_scalar_add(
                            o2T_sb[:, 0:512], o2_ps[:, 0:512], 0.0))
                        seq("dve", nc.vector.tensor_scalar_add(
                            o2T_sb[:, 512:PW], o2_ps[:, 512:1024], 0.0))
                if i >= 1:
                    o2_block(ct, i - 1)
                if g == 0:
                    # x4-7 staged through B (free until o2_block(0,0))
                    for b in range(4, 8):
                        ptr(x_sb, b, o2_ps[:, b * P:(b + 1) * P])
                    drain4("dve", xT, 4, o2_ps[:, 512:1024])
                if 4 <= g < 8:
                    late_ptr(g - 4)
                for (p_, j_, tbs) in o1_sched.get(g, []):
                    o1_chunk(p_, j_, tbs)
                if g in o1_drain_sched:
                    o1_drain(*o1_drain_sched[g])
                # l2 via 4-xbar groups: TT1, TT2 (bf16 pair adds, 2x DVE
                # mode), TT3 (tree root), short reduce
                if g >= 4:
                    pp, ss = divmod(g - 4, 16)
                    kk, ph = divmod(ss, 4)
                    if kk < 4:
                        if ph == 0:
                            l2_tt(pp, l2t1, 4 * kk, 4 * kk + 1)
                        elif ph == 1:
                            l2_tt(pp, l2t2, 4 * kk + 2, 4 * kk + 3)
                        elif ph == 2:
                            l2_tt(pp, l2t1, l2t1, l2t2)
                            l2_red(pp, kk)
                if g == 20:
                    l2_merge(0)
                if g == 21:
                    l1_chunk(0)
                    gate(2, (0, 1))
                elif g == 22:
                    gate(2, (2, 3))
                elif g == 23:
                    gate(2, (4, 5))
                elif g == 24:
                    gate(2, (6, 7))
                if g == 25:
                    l1_chunk(1)
                if g == 28:
                    gate(1, (0, 1, 2, 3))
                if g == 29:
                    l1_chunk(2)
                    store(0)
                exp_block(ct, i)
                xbar(ct, i)
                # lower-priority sync-queue work goes AFTER the slot's E-xbar
                # so the xbar pipeline never waits behind epilogue transposes
                if g == 18:
                    epi_xbar2(0)
                if g == 27:
                    epi_xbar1(0)
                if g == 31:
                    epi_xbar1(1)
                    l2_tt(1, l2t1, 12, 13)

            # ---- tail ----
            # PE: close o2, then remaining o1 chunks ((1,3) waits on the
            # last xbars' data)
            o2_block(1, NB - 1)
            o1_chunk(1, 2, (4, 5))
            o1_chunk(1, 2, (6, 7))
            o1_chunk(1, 3, range(8))                    # s 1536:2048
            # ACT (idle after last exp): o2 drain, then its transpose on sync
            with nc.allow_low_precision(reason="bf16 o2 staging"):
                seq("act", nc.scalar.copy(o2T_sb[:, PW:S], o2_ps[:]))
            epi_xbar2(1, q="sp")
            # DVE chain: k1 gates, quick drain of (1,2), l1/l2 finals,
            # drain of (1,3); epi transposes emitted right after each drain
            o1_drain(1, 2)
            epi_xbar1(2)
            l1_chunk(3)
            l2_tt(1, l2t2, 14, 15)
            l2_tt(1, l2t1, l2t1, l2t2)
            l2_red(1, 3)
            l2_merge(1)
            # o2 gates for the whole second half go as soon as r2 is ready
            gate(2, (8, 9, 10, 11))
            gate(2, (12, 13, 14, 15))
            gate(1, (4, 5, 6, 7))
            store(1)
            # (1,3) drain + o1N transpose + gates pipelined in 2-block halves
            o1_drain(1, 3, cols=(0, 256))
            epi_xbar1(3, h=0, q="act")
            o1_drain(1, 3, cols=(256, 512))
            epi_xbar1(3, h=1, q="sp")
            gate(1, (8, 9, 10, 11))
            store(2)
            gate(1, (12, 13))
            gate(1, (14, 15))
            store(3)

    nc.compile()
    return nc


def _get_nc():
    global _NC_CACHE
    if _NC_CACHE is None:
        nc = bacc.Bacc("TRN2", target_bir_lowering=False, debug=False,
                       num_devices=B)
        _NC_CACHE = _build_program(nc)
    return _NC_CACHE


def kernel(x, y):
    global LAST_EXEC_NS
    nc = _get_nc()
    x = np.asarray(x, dtype=np.float32)
    y = np.asarray(y, dtype=np.float32)
    in_maps = [
        {"x": np.ascontiguousarray(x[b]), "y": np.ascontiguousarray(y[b])}
        for b in range(B)
    ]
    trace = bool(int(os.environ.get("KERNEL_TRACE", "0")))
    res = run_bass_kernel_spmd(nc, in_maps, list(range(B)), trace=trace)
    LAST_EXEC_NS = res.exec_time_ns
    return np.stack([res.results[b]["out"] for b in range(B)], axis=0)


# revision 72
# speedup vs baseline: 1.0577x; 1.0577x over previous
"""BiModal attention kernel for Trainium2 (8 NeuronCores, data-parallel over batch).

Per core (one batch b): x, y: [2048, 128] fp32.
  S = x @ y.T                    (float32r matmuls, [2048, 2048])
  E = exp(S)                     (unshifted; softmax is shift-invariant and
                                  |S| <~ 67 so exp stays in fp32/bf16 range)
  a1 = (E @ y) / rowsum(E) * x
  a2 = (E.T @ x) / colsum(E) * y
  out = concat([a1, a2], -1)     ([2048, 256])

Layout: rows are relabeled r = 16*p + b (p = SBUF partition, b = block index)
so every DRAM transfer is contiguous per partition; the relabeling is applied
consistently to s and t everywhere, so the math is unchanged.

v8: uniform software pipeline. One slot per (panel ct, row block i), 32 slots.
Per slot: PE runs S (2x512 f32r) + o2 (2x512 bf16, one slot behind exp) + a
2-tb o1 chunk (512 s-cols); ACT runs exp (1 slot behind S) with l1 accum;
sync issues the E->ET xbar right after exp; DVE reduces that xbar's 8x128 ET
slice into l2 partials one slot later and handles PSUM drains; GpSimd does
bf16 operand copies, all output gating, and the output stores. o1 quarter
(p,j) runs as 2-tb chunks in slots 16p+4j+5..+8 (ET available 2 slots after
the quarter's last xbar), drain at +9, except (1,3) which runs in the tail
as two 8-tb x 256-col slices chasing the last xbars. Epilogue transposes are
quarter-granular ([128,512] via DMA xbar) and split across sync (o1 side)
and ACT (o2 side). Stores are combined [128, 4, 256] (o1|o2 halves gated
into one staging tile) so DRAM lines are 1KB contiguous.
PSUM: sA 2x[128,1024] (S rotation), o2_ps [128,1024] (panel accum), c_ps
[128,1024] (o1 quarter halves, alternating); prologue transposes stage
through all four before their owners need them.
"""
import sys

sys.path.insert(0, "/opt/trn_rl_repo")

import os
import numpy as np

import concourse.bass as bass
import concourse.mybir as mybir
import concourse.tile as tile
from concourse.tile_rust import add_dep_helper
from concourse import bacc
from concourse.bass_utils import run_bass_kernel_spmd
from concourse.masks import make_identity

f32 = mybir.dt.float32
f32r = mybir.dt.float32r
bf16 = mybir.dt.bfloat16

B = 8
S = 2048
D = 128
P = 128
NB = S // P          # 16 row/col blocks
NP = 2               # column panels
PW = S // NP         # panel width (1024)
PB = PW // P         # blocks per panel (8)

_NC_CACHE = None
LAST_EXEC_NS = None


def _build_program(nc):
    x_d = nc.dram_tensor("x", [S, D], f32, kind="ExternalInput").ap()
    y_d = nc.dram_tensor("y", [S, D], f32, kind="ExternalInput").ap()
    out_d = nc.dram_tensor("out", [S, 2 * D], f32, kind="ExternalOutput").ap()

    x_dv = x_d.rearrange("(p b) d -> p b d", p=P)      # [128, 16, 128]
    y_dv = y_d.rearrange("(p b) d -> p b d", p=P)
    out_dv = out_d.rearrange("(p b) c -> p b c", p=P)  # [128, 16, 256]

    Exp = mybir.ActivationFunctionType.Exp
    MUL = mybir.AluOpType.mult
    ADD = mybir.AluOpType.add
    AX = mybir.AxisListType.X

    last = {}

    def seq(key, inst):
        prev = last.get(key)
        if prev is not None:
            add_dep_helper(inst.ins, prev.ins, sync=False, reason="order")
        last[key] = inst
        return inst

    def xdep(inst, *writers):
        # Explicit RAW dep on DMA-transpose writes: the tile framework does
        # not track dma_start_transpose OUTPUT regions, so every consumer of
        # a transposed tile must be ordered manually.
        for w in writers:
            if w is None:
                continue
            for wi in (w if isinstance(w, list) else [w]):
                add_dep_helper(inst.ins, wi.ins, sync=True, reason="xbar-out")
        return inst

    with tile.TileContext(nc) as tc:
        with (
            tc.tile_pool(name="sb", bufs=1) as sb,
            tc.tile_pool(name="stg", bufs=4) as stg,
            tc.tile_pool(name="ps", bufs=1, space="PSUM") as ps,
        ):
            # ---- persistent SBUF tensors ----
            x_sb = sb.tile([P, NB, D], f32, tag="x_sb")
            y_sb = sb.tile([P, NB, D], f32, tag="y_sb")
            xT = sb.tile([P, NB, P], f32r, tag="xT")       # [d, sb, sp]
            yT = sb.tile([P, NB, P], f32r, tag="yT")       # [d, tb, tp]
            x_hi = sb.tile([P, NB, D], bf16, tag="x_hi")
            y_hi = sb.tile([P, NB, D], bf16, tag="y_hi")
            E = sb.tile([P, NB, S], bf16, tag="E")         # [sp, sb, t-pos]
            ET = sb.tile([P, NB, S], bf16, tag="ET")       # [tp, tb, s-pos]
            o1T_sb = sb.tile([P, S], bf16, tag="o1T")      # [d, s-pos]
            o2T_sb = sb.tile([P, S], bf16, tag="o2T")      # [d, t-pos]
            o1N = sb.tile([P, NB, D], bf16, tag="o1N")     # [sp, sb, d]
            o2N = sb.tile([P, NB, D], bf16, tag="o2N")     # [tp, tb, d]
            ident = sb.tile([P, P], f32, tag="ident")
            l1p = sb.tile([P, 2 * NB], f32, tag="l1p")     # [sp, 2*i+ct]
            l1c = sb.tile([P, NB], f32, tag="l1c")
            r1 = sb.tile([P, NB], f32, tag="r1")
            l2p = sb.tile([P, NB, 4], f32, tag="l2p")      # [tp, tb, group]
            l2t1 = sb.tile([P, PB, P], bf16, tag="l2t1")   # TT-tree tmps
            l2t2 = sb.tile([P, PB, P], bf16, tag="l2t2")
            l2 = sb.tile([P, NB], f32, tag="l2")
            r2 = sb.tile([P, NB], f32, tag="r2")

            make_identity(nc, ident[:])

            # ---- PSUM tiles (8 banks) ----
            sA = [ps.tile([P, PW], f32, tag="A0", name="sA0"),
                  ps.tile([P, PW], f32, tag="A1", name="sA1")]
            o2_ps = ps.tile([P, PW], f32, tag="B", name="o2_ps")
            c_ps = ps.tile([P, PW], f32, tag="C", name="c_ps")

            yT_f = yT[:].rearrange("p b d -> p (b d)")

            # ---- loads on 3 queue rings (~100GB/s each). The sync ring must
            # stay short: its later transfers would delay the first E-xbars.
            # First-S deps (x0, y0-7) land first.
            # first-needed chunks via gpsimd SWDGE (its queue is live at t~0;
            # the sync/scalar sequencers take ~7us to boot); the sync ring
            # stays clear so the first E-xbar transfer isn't queued behind
            # load traffic
            seq("sp", nc.sync.dma_start(x_sb[:, 0:4], x_dv[:, 0:4]))
            seq("sp", nc.sync.dma_start(y_sb[:, 0:4], y_dv[:, 0:4]))
            seq("act", nc.scalar.dma_start(y_sb[:, 4:8], y_dv[:, 4:8]))
            seq("act", nc.scalar.dma_start(x_sb[:, 4:8], x_dv[:, 4:8]))
            seq("act", nc.scalar.dma_start(y_sb[:, 8:16], y_dv[:, 8:16]))
            seq("act", nc.scalar.dma_start(x_sb[:, 8:16], x_dv[:, 8:16]))

            # ---- emission helpers ----
            def ptr(v_sb, b, pslice):
                seq("pe", nc.tensor.transpose(pslice, v_sb[:, b, :], ident[:]))

            def drain4(q, dst, b0, src, n=4):
                # [128, n*128] PSUM -> SBUF f32r
                eng = {"act": nc.scalar, "dve": nc.vector}[q]
                if q == "act":
                    seq(q, eng.copy(dst[:, b0:b0 + n, :], src))
                else:
                    seq(q, eng.tensor_scalar_add(dst[:, b0:b0 + n, :], src, 0.0))

            def conv(q, dst, src, b0, n):
                # f32 -> bf16 operand copies; "gp" uses the Pool engine's
                # casting DMA (software DGE) to keep DVE/ACT free
                if q == "gp":
                    seq(q, nc.gpsimd.dma_start(
                        dst[:, b0:b0 + n, :], src[:, b0:b0 + n, :]))
                elif q == "act":
                    seq(q, nc.scalar.copy(dst[:, b0:b0 + n, :],
                                          src[:, b0:b0 + n, :]))
                else:
                    seq(q, nc.vector.tensor_scalar_add(
                        dst[:, b0:b0 + n, :], src[:, b0:b0 + n, :], 0.0))

            def s_block(ct, i):
                c0 = ct * PW
                for q in range(2):
                    seq("pe", nc.tensor.matmul(
                        sA[i % 2][:, q * 512:(q + 1) * 512], xT[:, i, :],
                        yT_f[:, c0 + q * 512:c0 + (q + 1) * 512],
                        start=True, stop=True))

            def exp_block(ct, i):
                c0 = ct * PW
                idx = 2 * i + ct
                seq("act", nc.scalar.activation(
                    E[:, i, c0:c0 + PW], sA[i % 2][:], Exp,
                    accum_out=l1p[:, idx:idx + 1]))

            xbar_inst = {}
            epi1_inst = {}
            epi2_inst = {}

            def xbar(ct, i):
                c0 = ct * PW
                xbar_inst[(ct, i)] = seq("sp", nc.sync.dma_start_transpose(
                    ET[:, ct * PB:(ct + 1) * PB, i * P:(i + 1) * P],
                    E[:, i, c0:c0 + PW]))

            def o2_block(ct, i):
                c0 = ct * PW
                for q in range(2):
                    seq("pe", nc.tensor.matmul(
                        o2_ps[:, q * 512:(q + 1) * 512], x_hi[:, i, :],
                        E[:, i, c0 + q * 512:c0 + (q + 1) * 512],
                        start=(i == 0), stop=(i == NB - 1)))

            def o1_chunk(p, j, tbs):
                # o1 quarter (panel p, s-cols 512j..512j+512): contract tbs.
                # NOTE: start=True zeroes the whole 2KB psum bank, so each
                # bank half carries exactly one open group at a time.
                # Only the quarter's first matmul needs the xbar deps: the
                # rest follow in PE queue order.
                half = j % 2
                cb = c_ps[:, half * 512:half * 512 + 512]
                for tb in tbs:
                    inst = seq("pe", nc.tensor.matmul(
                        cb, y_hi[:, p * PB + tb, :],
                        ET[:, p * PB + tb, j * 512:(j + 1) * 512],
                        start=(tb == 0), stop=(tb == PB - 1)))
                    if tb == 0:
                        xdep(inst, *(xbar_inst.get((p, 4 * j + q))
                                     for q in range(4)))

            def o1_drain(p, j, cols=None):
                # both panels drain on DVE (an ACT copy here would stall the
                # exp cadence, which has no slack). cols slices the drain
                # (safe: reads only; no psum zero-region hazard)
                half = j % 2
                lo, hi = (0, 512) if cols is None else cols
                cb = c_ps[:, half * 512 + lo:half * 512 + hi]
                dst = o1T_sb[:, j * 512 + lo:j * 512 + hi]
                with nc.allow_low_precision(reason="bf16 o1 staging"):
                    if p == 0:
                        seq("dve", nc.vector.tensor_scalar_add(dst, cb, 0.0))
                    else:
                        seq("dve", nc.vector.tensor_tensor(dst, cb, dst,
                                                           op=ADD))

            def l2_tt(ct, dst, ia, ib):
                # pairwise bf16 add of two landed ET slices (DVE 2x mode);
                # exact sum association, one bf16 rounding per level
                tbs = slice(ct * PB, (ct + 1) * PB)
                with nc.allow_low_precision(reason="bf16 l2 tree"):
                    if isinstance(ia, int):
                        inst = seq("dve", nc.vector.tensor_tensor(
                            dst[:], ET[:, tbs, ia * P:(ia + 1) * P],
                            ET[:, tbs, ib * P:(ib + 1) * P], op=ADD))
                        xdep(inst, xbar_inst.get((ct, ia)),
                             xbar_inst.get((ct, ib)))
                    else:
                        seq("dve", nc.vector.tensor_tensor(
                            dst[:], ia[:], ib[:], op=ADD))

            def l2_red(ct, k):
                # reduce the group's tree root -> l2p[:, tbs, k]
                seq("dve", nc.vector.tensor_reduce(
                    l2p[:, ct * PB:(ct + 1) * PB, k:k + 1],
                    l2t1[:], axis=AX, op=ADD))

            def l2_merge(p):
                seq("dve", nc.vector.tensor_reduce(
                    l2[:, p * PB:(p + 1) * PB],
                    l2p[:, p * PB:(p + 1) * PB, :], axis=AX, op=ADD))
                seq("dve", nc.vector.reciprocal(r2[:, p * PB:(p + 1) * PB],
                                                l2[:, p * PB:(p + 1) * PB]))

            def l1_chunk(k):
                src = l1p[:, 8 * k:8 * k + 8].rearrange("p (i c) -> p i c", c=2)
                seq("dve", nc.vector.tensor_reduce(l1c[:, 4 * k:4 * k + 4], src,
                                                   axis=AX, op=ADD))
                seq("dve", nc.vector.reciprocal(r1[:, 4 * k:4 * k + 4],
                                                l1c[:, 4 * k:4 * k + 4]))

            def epi_xbar1(j, h=None, q="sp"):
                # o1T quarter [128, 512] -> o1N blocks 4j..4j+3; h selects a
                # [128, 256] half (2 blocks) for the pipelined tail
                eng = {"act": nc.scalar, "sp": nc.sync}[q]
                if h is None:
                    inst = seq(q, eng.dma_start_transpose(
                        o1N[:, 4 * j:4 * j + 4, :],
                        o1T_sb[:, j * 512:(j + 1) * 512]))
                    epi1_inst[(j, 0)] = epi1_inst[(j, 1)] = inst
                else:
                    inst = seq(q, eng.dma_start_transpose(
                        o1N[:, 4 * j + 2 * h:4 * j + 2 * h + 2, :],
                        o1T_sb[:, j * 512 + 256 * h:j * 512 + 256 * h + 256]))
                    epi1_inst[(j, h)] = inst

            def epi_xbar2(half, q="act"):
                # o2T half [128, 1024] -> o2N blocks 8h..8h+7
                eng = {"act": nc.scalar, "sp": nc.sync}[q]
                epi2_inst[half] = seq(q, eng.dma_start_transpose(
                    o2N[:, half * PB:(half + 1) * PB, :],
                    o2T_sb[:, half * PW:(half + 1) * PW]))

            st_tiles = {}

            def gate(which, blks):
                # gate blocks of o1 (which=1) or o2 (which=2) into staging
                k = blks[0] // 4
                if k not in st_tiles:
                    st_tiles[k] = stg.tile([P, 4, 2 * D], f32, tag="st",
                                           name=f"st{k}")
                st = st_tiles[k]
                srcN = o1N if which == 1 else o2N
                rcp = r1 if which == 1 else r2
                gsrc = x_sb if which == 1 else y_sb
                col0 = 0 if which == 1 else D
                for blk in blks:
                    inst = seq("dve", nc.vector.scalar_tensor_tensor(
                        st[:, blk % 4, col0:col0 + D], srcN[:, blk, :],
                        rcp[:, blk:blk + 1], gsrc[:, blk, :],
                        op0=MUL, op1=MUL))
                    if which == 1:
                        xdep(inst, epi1_inst.get((blk // 4, (blk % 4) // 2)))
                    else:
                        xdep(inst, epi2_inst.get(blk // 8))

            def store(k):
                seq("gp", nc.gpsimd.dma_start(
                    out_dv[:, 4 * k:4 * k + 4, :], st_tiles[k][:]))

            # ---- prologue: minimal set for S(0,0) = xT[0] + yT[0:8];
            # everything else staged just-in-time during early slots ----
            ptr(x_sb, 0, sA[0][:, 0:P])          # x0 -> A0.lo[0:128]
            for b in range(1, 4):                # x1-3 -> A0.hi[512:896]
                ptr(x_sb, b, sA[0][:, (b + 3) * P:(b + 4) * P])
            for b in range(4):                   # y0-3 -> B[0:512]
                ptr(y_sb, b, o2_ps[:, b * P:(b + 1) * P])
            for b in range(4, 8):                # y4-7 -> A1[0:512]
                ptr(y_sb, b, sA[1][:, (b - 4) * P:(b - 3) * P])
            drain4("act", xT, 0, sA[0][:, 0:P], n=1)
            drain4("act", yT, 0, o2_ps[:, 0:512])
            drain4("act", yT, 4, sA[1][:, 0:512])
            drain4("dve", xT, 1, sA[0][:, 512:896], n=3)
            # bf16 copies (gpsimd casting DMAs), first-needed first
            conv("gp", x_hi, x_sb, 0, 4)
            conv("gp", y_hi, y_sb, 0, 4)
            conv("gp", x_hi, x_sb, 4, 4)
            conv("gp", y_hi, y_sb, 4, 4)
            conv("gp", x_hi, x_sb, 8, 8)
            conv("gp", y_hi, y_sb, 8, 8)

            # late transposes (blocks 8-15) through C halves, slots 4-7
            # (ordered to match when their load chunks land)
            def late_ptr(g):
                v_sb, b0, dst, half = (
                    (y_sb, 8, yT, 0), (y_sb, 12, yT, 0),
                    (x_sb, 8, xT, 1), (x_sb, 12, xT, 1))[g]
                for b in range(b0, b0 + 4):
                    ptr(v_sb, b, c_ps[:, half * 512 + (b - b0) * P:
                                      half * 512 + (b - b0 + 1) * P])
                drain4("dve", dst, b0, c_ps[:, half * 512:half * 512 + 512])

            # o1 chunk slot map: (p, j) occupies slots 16p+4j+6 .. +9,
            # 2 tbs per slot (1 slot of margin past the quarter's last xbar);
            # (1,2) spills 2 chunks to the tail, (1,3) runs fully in the tail.
            o1_sched = {}
            for p_ in range(2):
                for j_ in range(4):
                    if (p_, j_) == (1, 3):
                        continue
                    base = 16 * p_ + 4 * j_ + 6
                    for c_ in range(4):
                        g_ = base + c_
                        if g_ >= 32:
                            continue
                        o1_sched.setdefault(g_, []).append(
                            (p_, j_, (2 * c_, 2 * c_ + 1)))
            o1_drain_sched = {16 * p_ + 4 * j_ + 10: (p_, j_)
                              for p_ in range(2) for j_ in range(4)
                              if 16 * p_ + 4 * j_ + 10 < 32}

            # ---- main loop: 32 uniform slots ----
            for g in range(32):
                ct, i = divmod(g, 16)
                s_block(ct, i)
                if g == 16:
                    # close panel-0 o2 and drain it BEFORE o2_block(1, 0)
                    # opens the new accumulation group on the same banks;
                    # drain in bank halves so the WAR releases per-bank
                    o2_block(0, NB - 1)
                    with nc.allow_low_precision(reason="bf16 o2 staging"):
                        seq("dve", nc.vector.tensor_scalar_add(
                            o2T_sb[:, 0:512], o2_ps[:, 0:512], 0.0))
                        seq("dve", nc.vector.tensor_scalar_add(
                            o2T_sb[:, 512:PW], o2_ps[:, 512:1024], 0.0))
                if i >= 1:
                    o2_block(ct, i - 1)
                if g == 0:
                    # x4-7 staged through B (free until o2_block(0,0))
                    for b in range(4, 8):
                        ptr(x_sb, b, o2_ps[:, b * P:(b + 1) * P])
                    drain4("dve", xT, 4, o2_ps[:, 512:1024])
                if 4 <= g < 8:
                    late_ptr(g - 4)
                for (p_, j_, tbs) in o1_sched.get(g, []):
                    o1_chunk(p_, j_, tbs)
                if g in o1_drain_sched:
                    o1_drain(*o1_drain_sched[g])
                # l2 via 4-xbar groups: TT1, TT2 (bf16 pair adds, 2x DVE
                # mode), TT3 (tree root), short reduce
                if g >= 4:
                    pp, ss = divmod(g - 4, 16)
                    kk, ph = divmod(ss, 4)
                    if kk < 4:
                        if ph == 0:
                            l2_tt(pp, l2t1, 4 * kk, 4 * kk + 1)
                        elif ph == 1:
                            l2_tt(pp, l2t2, 4 * kk + 2, 4 * kk + 3)
                        elif ph == 2:
                            l2_tt(pp, l2t1, l2t1, l2t2)
                            l2_red(pp, kk)
                if g == 20:
                    l2_merge(0)
                if g == 21:
                    l1_chunk(0)
                    gate(2, (0, 1))
                elif g == 22:
                    gate(2, (2, 3))
                elif g == 23:
                    gate(2, (4, 5))
                elif g == 24:
                    gate(2, (6, 7))
                if g == 25:
                    l1_chunk(1)
                if g == 28:
                    gate(1, (0, 1, 2, 3))
                if g == 29:
                    l1_chunk(2)
                    store(0)
                exp_block(ct, i)
                xbar(ct, i)
                # lower-priority sync-queue work goes AFTER the slot's E-xbar
                # so the xbar pipeline never waits behind epilogue transposes
                if g == 18:
                    epi_xbar2(0)
                if g == 27:
                    epi_xbar1(0)
                if g == 31:
                    epi_xbar1(1)
                    l2_tt(1, l2t1, 12, 13)

            # ---- tail ----
            # PE: close o2, then remaining o1 chunks ((1,3) waits on the
            # last xbars' data)
            o2_block(1, NB - 1)
            o1_chunk(1, 2, (4, 5))
            o1_chunk(1, 2, (6, 7))
            o1_chunk(1, 3, range(8))                    # s 1536:2048
            # ACT (idle after last exp): o2 drain, then its transpose on sync
            with nc.allow_low_precision(reason="bf16 o2 staging"):
                seq("act", nc.scalar.copy(o2T_sb[:, PW:S], o2_ps[:]))
            epi_xbar2(1, q="sp")
            # DVE chain: k1 gates, quick drain of (1,2), l1/l2 finals,
            # drain of (1,3); epi transposes emitted right after each drain
            gate(1, (4, 5, 6, 7))
            store(1)
            o1_drain(1, 2)
            epi_xbar1(2)
            l1_chunk(3)
            l2_tt(1, l2t2, 14, 15)
            l2_tt(1, l2t1, l2t1, l2t2)
            l2_red(1, 3)
            l2_merge(1)
            # o2 gates for the whole second half go as soon as r2 is ready
            gate(2, (8, 9, 10, 11))
            gate(2, (12, 13, 14, 15))
            # (1,3) drain + o1N transpose + gates pipelined in 2-block halves
            o1_drain(1, 3, cols=(0, 256))
            epi_xbar1(3, h=0, q="act")
            o1_drain(1, 3, cols=(256, 512))
            epi_xbar1(3, h=1, q="sp")
            gate(1, (8, 9, 10, 11))
            store(2)
            gate(1, (12, 13))
            gate(1, (14, 15))
            store(3)

    nc.compile()
    return nc


def _get_nc():
    global _NC_CACHE
    if _NC_CACHE is None:
        nc = bacc.Bacc("TRN2", target_bir_lowering=False, debug=False,
                       num_devices=B)
        _NC_CACHE = _build_program(nc)
    return _NC_CACHE


def kernel(x, y):
    global LAST_EXEC_NS
    nc = _get_nc()
    x = np.asarray(x, dtype=np.float32)
    y = np.asarray(y, dtype=np.float32)
    in_maps = [
        {"x": np.ascontiguousarray(x[b]), "y": np.ascontiguousarray(y[b])}
        for b in range(B)
    ]
    trace = bool(int(os.environ.get("KERNEL_TRACE", "0")))
    res = run_bass_kernel_spmd(nc, in_maps, list(range(B)), trace=trace)
    LAST_EXEC_NS = res.exec_time_ns
    return np.stack([res.results[b]["out"] for b in range(B)], axis=0)
